# revision 1
# baseline (speedup 1.0000x reference)
"""Trainium2 Bass kernel for MultiHeadedAttentionSANM (B=16, T=1024, F=512, H=4, K=11).

Sharding: data-parallel over batch across 8 NeuronCores (2 batch items per
core), no collectives. Host pre-transposes x to feature-major layout and
re-transposes the output; the mask is exploited as a valid-prefix (first
`nv` frames valid), detected on host.

Per-core dataflow (fp16 operands, fp32 PSUM accumulation everywhere):
  xT (feat x tok) --w_qkv--> qT,kT feature-major; v row-major; vmT feature-major
  scoresT[tk,tq] = kT_h.T @ qT_h   (keys on partitions; 1/sqrt(dk) scale and
                                    a constant bias fold into the Exp act)
  expT = Exp(scale*scoresT - 3)    (ScalarE, PSUM->SBUF)
  denom = ones.T @ expT            (PE accumulate, M=1)
  ctxT_h = v_h.T @ expT            (PE accumulate)
  1/denom = Exp(-Ln(denom))        (ScalarE; vector.reciprocal is 8 cyc/elem)
  ctx_norm = ctxT * bcast(1/denom) (K=1 ones matmul broadcast + DVE mult)
  att_outT = w_out.T @ ctx_norm    (feature-major out)
  fsmn: depthwise conv + residual as 12 accumulating diagonal matmuls on the
        PE (host-built diag weights; shifts are free AP offsets on the
        zero-padded vmT buffer; 12th tap = identity = residual)
  out = att_outT + fsmn            (valid frames); att_outT on the padded tail

The head loop is software-pipelined by one head (scores/exp of head h issue
before denominator/ctx of head h-1) to keep the PE dense and HAM-warm.
"""

import sys

sys.path.insert(0, "/opt/trn_rl_repo")

import numpy as np

import concourse.bass as bass
import concourse.mybir as mybir
import concourse.tile as tile
from concourse.bass_utils import run_bass_kernel_spmd

F32 = mybir.dt.float32
F32R = mybir.dt.float32r
FP16 = mybir.dt.float16

N_CORES = 8
B, T, F = 16, 1024, 512
H, DK = 4, 128
KERNEL = 11
LEFT_PAD = (KERNEL - 1) // 2  # 5
NTAP = KERNEL + 1  # 11 conv taps + identity (residual)
NB = B // N_CORES  # batch items per core
SCALE = DK ** -0.5
EXP_BIAS = -3.0  # constant shift inside exp; cancels in softmax normalization

PRECISION = "fp16"  # "fp16" | "mixed" (fp32r projections+scores)

Alu = mybir.AluOpType
Act = mybir.ActivationFunctionType


def _split_multiwaits(nc, max_waits=1):
    """walrus on this toolchain accepts at most one sync-wait command per
    instruction; split extras onto same-engine NoOps placed just before."""
    n_split = 0
    for fn in nc.m.functions:
        for bb in fn.blocks:
            out = []
            for inst in bb.instructions:
                si = inst.sync_info
                if si is not None and len(si.on_wait) > max_waits:
                    waits = list(si.on_wait)
                    for w in waits[:-max_waits]:
                        nop = mybir.InstNoOp(
                            name=nc.get_next_instruction_name(),
                            engine=inst.engine,
                            sync_info=mybir.SyncInfo(on_wait=[w], on_update=[]),
                            bass_nofuse=True,
                        )
                        out.append(nop)
                        n_split += 1
                    inst.sync_info = mybir.SyncInfo(
                        on_wait=waits[-max_waits:], on_update=list(si.on_update)
                    )
                out.append(inst)
            bb.instructions = out
    return n_split


def _ceil_div(a, b):
    return (a + b - 1) // b


def _n_chunks(n, c=512):
    out = []
    s = 0
    while s < n:
        out.append((s, min(c, n - s)))
        s += c
    return out


def _build(nv, use_bqkv, use_bout, prec):
    if prec == "fp16":
        DT_A = DT_B = FP16  # projection inputs / q,k
    elif prec == "mixed":
        DT_A = DT_B = F32R
    else:
        raise ValueError(prec)
    DT_C = FP16  # vrow, expT, ones for denom/ctx matmuls
    DT_E = FP16  # w_out, ctx
    DT_F = FP16  # padded vmT + diag weights (fsmn matmuls)
    DT_D = FP16  # 1/denom + ones row (broadcast matmul)

    nc = bass.Bass()

    DT_IN = FP16 if DT_A == FP16 else F32
    xT_p = nc.declare_dram_parameter("xT", [NB, F, T], DT_IN, isOutput=False)
    wqkv_p = nc.declare_dram_parameter("wqkv", [F, 3 * F], DT_IN, isOutput=False)
    wout_p = nc.declare_dram_parameter("wout", [F, F], FP16, isOutput=False)
    wdiag_p = nc.declare_dram_parameter(
        "wdiag", [128, F // 128, NTAP, 128], mybir.dt.float16, isOutput=False
    )
    wfsmn_p = nc.declare_dram_parameter("wfsmn", [128, 4, KERNEL], F32, isOutput=False)
    if use_bqkv:
        bqkv_p = nc.declare_dram_parameter("bqkv", [1, 3 * F], F32, isOutput=False)
    if use_bout:
        bout_p = nc.declare_dram_parameter("bout", [128, 4], F32, isOutput=False)
    out_p = nc.declare_dram_parameter("outT", [NB, F, T], F32, isOutput=True)

    nvt = _ceil_div(nv, 128)  # valid key tiles
    nv_part = nv - (nvt - 1) * 128  # rows in the last key tile
    if nv_part != 128:
        vcol_p = nc.declare_dram_parameter("vcol", [128, 1], F32, isOutput=False)

    FC = F // 128  # 4 feature chunks
    TP = T + KERNEL - 1  # padded fsmn time extent

    with tile.TileContext(nc) as tc:
        with (
            tc.tile_pool(name="consts", bufs=1) as consts,
            tc.tile_pool(name="stage", bufs=2) as stage,
            tc.tile_pool(name="xtr", bufs=2) as xtr,
            tc.tile_pool(name="peritem", bufs=1) as peritem,
            tc.tile_pool(name="peritem2", bufs=2) as peritem2,
            tc.tile_pool(name="expp", bufs=12) as expp,
            tc.tile_pool(name="smalls", bufs=2) as smalls,
            tc.tile_pool(name="accp", bufs=4) as accp,
            tc.tile_pool(name="finp", bufs=4) as finp,
            tc.tile_pool(name="ps_proj", bufs=2, space="PSUM") as ps_proj,
            tc.tile_pool(name="ps_s", bufs=2, space="PSUM") as ps_s,
            tc.tile_pool(name="ps_c", bufs=1, space="PSUM") as ps_c,
            tc.tile_pool(name="ps_d", bufs=1, space="PSUM") as ps_d,
        ):
            # ---- constants / weights ----
            wq_t = [consts.tile([128, 3 * F], DT_A, tag=f"wq{_ic}",
                                name=f"wq{_ic}") for _ic in range(FC)]
            wout_e = consts.tile([128, FC, F], DT_E, tag="wout")
            if DT_A == FP16:
                for ic in range(FC):
                    nc.sync.dma_start(
                        out=wq_t[ic],
                        in_=wqkv_p[ic * 128:(ic + 1) * 128, :],
                    )
            else:
                for ic in range(FC):
                    st1 = stage.tile([128, 1024], F32, tag="stg")
                    nc.sync.dma_start(
                        out=st1, in_=wqkv_p[ic * 128:(ic + 1) * 128, 0:1024]
                    )
                    nc.vector.tensor_copy(wq_t[ic][:, 0:1024], st1[:, :])
                    st2 = stage.tile([128, 1024], F32, tag="stg")
                    nc.sync.dma_start(
                        out=st2[:, 0:512],
                        in_=wqkv_p[ic * 128:(ic + 1) * 128, 1024:1536],
                    )
                    nc.vector.tensor_copy(wq_t[ic][:, 1024:1536], st2[:, 0:512])
            wdiag = consts.tile([128, FC, NTAP, 128], DT_F, tag="wdiag")
            wfsmn_t = consts.tile([128, FC, KERNEL], F32, tag="wfsmn")
            nc.sync.dma_start(out=wfsmn_t, in_=wfsmn_p[:, :, :])

            ones_col = consts.tile([128, 1], DT_C, tag="onescol")
            tmp_oc = consts.tile([128, 1], F32, tag="onescol_f")
            nc.vector.memset(tmp_oc, 1.0)
            nc.vector.tensor_copy(ones_col, tmp_oc)
            ones_row = consts.tile([1, 128], DT_D, tag="onesrow")
            tmp_or = consts.tile([1, 128], F32, tag="onesrow_f")
            nc.vector.memset(tmp_or, 1.0)
            nc.vector.tensor_copy(ones_row, tmp_or)
            expb = consts.tile([128, 1], F32, tag="expb")
            nc.vector.memset(expb, EXP_BIAS)
            if use_bqkv:
                ones_row512 = consts.tile([1, 512], DT_A, tag="onesrow512")
                tmp_o5 = consts.tile([1, 512], F32, tag="onesrow512_f")
                nc.vector.memset(tmp_o5, 1.0)
                nc.vector.tensor_copy(ones_row512, tmp_o5)
                bqkv_stage = consts.tile([1, 3 * F], F32, tag="bqkv_f")
                nc.sync.dma_start(out=bqkv_stage, in_=bqkv_p[:, :])
                bqkv_a = consts.tile([1, 3 * F], DT_A, tag="bqkv")
                nc.vector.tensor_copy(bqkv_a, bqkv_stage)
            if use_bout:
                bout_t = consts.tile([128, 4], F32, tag="bout")
                nc.sync.dma_start(out=bout_t, in_=bout_p[:, :])
            if nv_part != 128:
                vcol_t = consts.tile([128, 1], F32, tag="vcol")
                nc.sync.dma_start(out=vcol_t, in_=vcol_p[:, :])

            # persistent zero-padded vmT buffer (feature-major, shared items)
            vTp = peritem.tile([128, FC, TP], DT_F, tag="vTp")
            nc.vector.memset(vTp, 0.0)

            def bias_mm(psum_ap, oc_global, nsz):
                nc.tensor.matmul(
                    psum_ap,
                    bqkv_a[:, oc_global * 128:(oc_global + 1) * 128],
                    ones_row512[:, 0:nsz],
                    start=False,
                    stop=True,
                )

            def emit_load_proj(item):
                # ---- load xT ----
                xT_t = [xtr.tile([128, T], DT_A, tag=f"xT{_ic}",
                                 name=f"xT{_ic}_{item}") for _ic in range(FC)]
                if DT_A == FP16:
                    for ic in range(FC):
                        nc.scalar.dma_start(
                            out=xT_t[ic],
                            in_=xT_p[item, ic * 128:(ic + 1) * 128, :],
                        )
                else:
                    for ic in range(FC):
                        st = stage.tile([128, 1024], F32, tag="stg")
                        nc.sync.dma_start(
                            out=st, in_=xT_p[item, ic * 128:(ic + 1) * 128, :]
                        )
                        nc.vector.tensor_copy(xT_t[ic][:, :], st[:, :])
                if item == 0:
                    # late-needed weights load after the critical-path inputs
                    nc.sync.dma_start(
                        out=wout_e,
                        in_=wout_p.rearrange("(c p) o -> p c o", p=128),
                    )
                    nc.sync.dma_start(out=wdiag, in_=wdiag_p[:, :, :, :])

                # ---- projections ----
                qT = peritem2.tile([128, H, T], DT_B, tag="qT")
                kT = peritem2.tile([128, H, nvt * 128], DT_B, tag="kT")

                def fm_proj(dst, ocg, chunks):
                    """feature-major projection chunk group with lhsT reuse"""
                    pss = [ps_proj.tile([128, 512], F32, tag="proj",
                                        name=f"pjq{_i}")
                           for _i in range(len(chunks))]
                    for ic in range(FC):
                        for psx, (t0, tsz) in zip(pss, chunks):
                            nc.tensor.matmul(
                                psx[:, 0:tsz],
                                wq_t[ic][:, ocg * 128:(ocg + 1) * 128],
                                xT_t[ic][:, t0:t0 + tsz],
                                start=(ic == 0),
                                stop=(ic == FC - 1) and not use_bqkv,
                            )
                    for psx, (t0, tsz) in zip(pss, chunks):
                        if use_bqkv:
                            bias_mm(psx[:, 0:tsz], ocg, tsz)
                        nc.vector.tensor_copy(dst[:, t0:t0 + tsz], psx[:, 0:tsz])

                for h in range(H):
                    fm_proj(qT[:, h, :], h, _n_chunks(T))
                for h in range(H):
                    fm_proj(kT[:, h, :], FC + h, _n_chunks(nv))
                vrow = peritem2.tile([128, nvt, F], DT_C, tag="vrow")
                for tt in range(nvt):
                    trows = min(128, nv - tt * 128)
                    ps = ps_proj.tile([128, 512], F32, tag="proj")
                    for ic in range(FC):
                        nc.tensor.matmul(
                            ps[:trows, :],
                            xT_t[ic][:, tt * 128:tt * 128 + trows],
                            wq_t[ic][:, 2 * F:3 * F],
                            start=(ic == 0),
                            stop=(ic == FC - 1) and not use_bqkv,
                        )
                    if use_bqkv:
                        nc.tensor.matmul(
                            ps[:trows, :],
                            ones_row512[:, 0:trows],
                            bqkv_a[:, 2 * F:3 * F],
                            start=False,
                            stop=True,
                        )
                    nc.vector.tensor_copy(vrow[:trows, tt, :], ps[:trows, :])
                for cc in range(FC):
                    chunks = _n_chunks(nv)
                    pss = [ps_proj.tile([128, 512], F32, tag="proj",
                                        name=f"pjq{_i}")
                           for _i in range(len(chunks))]
                    for ic in range(FC):
                        for psx, (t0, tsz) in zip(pss, chunks):
                            nc.tensor.matmul(
                                psx[:, 0:tsz],
                                wq_t[ic][:, 2 * F + cc * 128:2 * F + (cc + 1) * 128],
                                xT_t[ic][:, t0:t0 + tsz],
                                start=(ic == 0),
                                stop=(ic == FC - 1) and not use_bqkv,
                            )
                    for psx, (t0, tsz) in zip(pss, chunks):
                        if use_bqkv:
                            bias_mm(psx[:, 0:tsz], 2 * FC + cc, tsz)
                        nc.vector.tensor_copy(
                            vTp[:, cc, LEFT_PAD + t0:LEFT_PAD + t0 + tsz],
                            psx[:, 0:tsz],
                        )

                return qT, kT, vrow

            def emit_attention(item, qT, kT, vrow):
                # ---- attention, software-pipelined by one head; the fsmn
                # diag-matmul conv for chunk (step-1) rides in each slot ----
                ctx = peritem2.tile([128, H, T], DT_E, tag="ctx")
                prev = None
                faccs = []
                for step in range(H + 1):
                    if prev is not None:
                        # denominator of the previous head first so its
                        # Ln/Exp runs on ACT while this head's scores stream
                        ph, pets, pes = prev
                        dn = ps_d.tile([1, 1024], F32, tag="dnb")
                        for q0, qsz in _n_chunks(T):
                            nc.tensor.matmul(
                                dn[:, q0:q0 + qsz],
                                ones_col[:, :],
                                pes[:, q0:q0 + qsz],
                                start=True,
                                stop=True,
                            )
                        rec_f = smalls.tile([1, 1024], F32, tag="rec_f")
                        nc.scalar.activation(rec_f, dn[:, :], Act.Ln)
                        rec_r = smalls.tile([1, 1024], DT_D, tag="rec_r")
                        nc.scalar.activation(rec_r, rec_f[:, :], Act.Exp, scale=-1.0)
                    if step < H:
                        h = step
                        ets = []
                        es = None
                        for tkt in range(nvt):
                            krows = min(128, nv - tkt * 128)
                            et = expp.tile([128, 1024], DT_C, tag="expT")
                            for q0, qsz in _n_chunks(T):
                                sps = ps_s.tile([128, 512], F32, tag="scores")
                                nc.tensor.matmul(
                                    sps[:krows, 0:qsz],
                                    kT[:, h, tkt * 128:tkt * 128 + krows],
                                    qT[:, h, q0:q0 + qsz],
                                    start=True,
                                    stop=True,
                                )
                                nc.scalar.activation(
                                    et[:krows, q0:q0 + qsz], sps[:krows, 0:qsz],
                                    Act.Exp, bias=expb[:krows, 0:1], scale=SCALE,
                                )
                            if krows != 128:
                                nc.vector.tensor_scalar_mul(
                                    et[:, :], et[:, :], vcol_t[:, 0:1]
                                )
                            ets.append(et)
                            # incremental partition-wise exp sum on DVE
                            if tkt == 1:
                                es = smalls.tile([128, 1024], DT_C, tag="esum")
                                nc.vector.tensor_tensor(
                                    out=es, in0=ets[0][:, :], in1=et[:, :],
                                    op=Alu.add,
                                )
                            elif tkt >= 2:
                                nc.vector.tensor_tensor(
                                    out=es, in0=es, in1=et[:, :], op=Alu.add,
                                )
                        if es is None:
                            es = ets[0]
                    if prev is not None:
                        cps = ps_c.tile([128, 1024], F32, tag="ctx")
                        for tkt in range(nvt):
                            krows = min(128, nv - tkt * 128)
                            et = pets[tkt]
                            for q0, qsz in _n_chunks(T):
                                nc.tensor.matmul(
                                    cps[:, q0:q0 + qsz],
                                    vrow[:krows, tkt, ph * 128:(ph + 1) * 128],
                                    et[:krows, q0:q0 + qsz],
                                    start=(tkt == 0),
                                    stop=(tkt == nvt - 1),
                                )
                        bps = ps_d.tile([128, 1024], F32, tag="dnb")
                        for q0, qsz in _n_chunks(T):
                            nc.tensor.matmul(
                                bps[:, q0:q0 + qsz],
                                ones_row[:, :],
                                rec_r[:, q0:q0 + qsz],
                                start=True,
                                stop=True,
                            )
                        bcs = smalls.tile([128, 1024], F32, tag="bcast")
                        nc.scalar.copy(bcs, bps[:, :])
                        nc.vector.tensor_tensor(
                            out=ctx[:, ph, :], in0=cps[:, :], in1=bcs[:, :],
                            op=Alu.mult,
                        )
                        # fsmn conv+residual for chunk ph: 12 accumulating
                        # diagonal matmuls per 512-chunk of valid frames
                        cc = ph
                        facc = accp.tile([128, nv], F32, tag="facc")
                        fchunks = _n_chunks(nv)
                        fpss = [ps_proj.tile([128, 512], F32, tag="proj",
                                             name=f"pjf{_i}")
                                for _i in range(len(fchunks))]
                        for jt in range(4, NTAP):
                            sh = jt if jt < KERNEL else LEFT_PAD
                            for fps, (t0, tsz) in zip(fpss, fchunks):
                                nc.tensor.matmul(
                                    fps[:, 0:tsz],
                                    wdiag[:, cc, jt, :],
                                    vTp[:, cc, t0 + sh:t0 + sh + tsz],
                                    start=(jt == 4),
                                    stop=(jt == NTAP - 1),
                                )
                        # taps 0..3 on VectorE into an fp32 partial
                        facc2 = accp.tile([128, nv], F32, tag="facc2")
                        nc.vector.tensor_scalar_mul(
                            facc2, vTp[:, cc, 0:nv], wfsmn_t[:, cc, 0:1]
                        )
                        for j in range(1, 4):
                            nc.vector.scalar_tensor_tensor(
                                out=facc2,
                                in0=vTp[:, cc, j:j + nv],
                                scalar=wfsmn_t[:, cc, j:j + 1],
                                in1=facc2,
                                op0=Alu.mult,
                                op1=Alu.add,
                            )
                        for fps, (t0, tsz) in zip(fpss, fchunks):
                            nc.vector.scalar_tensor_tensor(
                                out=facc[:, t0:t0 + tsz],
                                in0=facc2[:, t0:t0 + tsz],
                                scalar=(bout_t[:, cc:cc + 1] if use_bout else 0.0),
                                in1=fps[:, 0:tsz],
                                op0=Alu.add,
                                op1=Alu.add,
                            )
                        faccs.append(facc)
                    if step < H:
                        prev = (step, ets, es)

                return ctx, faccs

            def emit_outproj(item, ctx, faccs):
                # ---- out projection + final combine ----
                for oc in range(FC):
                    fin = finp.tile([128, T], F32, tag="final")
                    ochunks = _n_chunks(T)
                    opss = [ps_proj.tile([128, 512], F32, tag="proj",
                                         name=f"pjo{_i}")
                            for _i in range(len(ochunks))]
                    for fc in range(FC):
                        for ps, (q0, qsz) in zip(opss, ochunks):
                            nc.tensor.matmul(
                                ps[:, 0:qsz],
                                wout_e[:, fc, oc * 128:(oc + 1) * 128],
                                ctx[:, fc, q0:q0 + qsz],
                                start=(fc == 0),
                                stop=(fc == FC - 1),
                            )
                    for ps, (q0, qsz) in zip(opss, ochunks):
                        if q0 < nv:
                            vsz = min(qsz, nv - q0)
                            nc.vector.scalar_tensor_tensor(
                                out=fin[:, q0:q0 + vsz],
                                in0=faccs[oc][:, q0:q0 + vsz],
                                scalar=1.0,
                                in1=ps[:, 0:vsz],
                                op0=Alu.bypass,
                                op1=Alu.add,
                            )
                        if q0 + qsz > nv:
                            t0 = max(q0, nv)
                            if use_bout:
                                nc.vector.tensor_scalar_add(
                                    fin[:, t0:q0 + qsz],
                                    ps[:, t0 - q0:qsz],
                                    bout_t[:, oc:oc + 1],
                                )
                            else:
                                nc.scalar.copy(
                                    fin[:, t0:q0 + qsz], ps[:, t0 - q0:qsz]
                                )
                    nc.sync.dma_start(
                        out=out_p[item, oc * 128:(oc + 1) * 128, :], in_=fin
                    )

            # interleave items so the PE never waits at phase boundaries:
            # item1 projections fill the gap while item0's last normalize
            # chain completes before its out-projection
            p0 = emit_load_proj(0)
            a0 = emit_attention(0, *p0)
            if NB > 1:
                p1 = emit_load_proj(1)
                emit_outproj(0, *a0)
                a1 = emit_attention(1, *p1)
                emit_outproj(1, *a1)
            else:
                emit_outproj(0, *a0)

    _split_multiwaits(nc)
    return nc


_cache = {}


def _get_nc(nv, use_bqkv, use_bout, prec=None):
    prec = prec or PRECISION
    key = (nv, use_bqkv, use_bout, prec)
    if key not in _cache:
        _cache[key] = _build(nv, use_bqkv, use_bout, prec)
    return _cache[key]


def _make_wdiag(w_fsmn):
    """(128, FC, NTAP, 128) fp16: per chunk, 11 diag(w[:, j]) + identity."""
    FC = F // 128
    wd = np.zeros((128, FC, NTAP, 128), np.float16)
    idx = np.arange(128)
    for cc in range(FC):
        for j in range(KERNEL):
            wd[idx, cc, j, idx] = w_fsmn[cc * 128 + idx, j].astype(np.float16)
        wd[idx, cc, KERNEL, idx] = 1.0
    return wd


def kernel(x, mask, w_qkv, b_qkv, w_out, b_out, w_fsmn):
    x = np.asarray(x, dtype=np.float32)
    mask = np.asarray(mask, dtype=np.float32)
    w_qkv = np.asarray(w_qkv, dtype=np.float32)
    b_qkv = np.asarray(b_qkv, dtype=np.float32)
    w_out = np.asarray(w_out, dtype=np.float32)
    b_out = np.asarray(b_out, dtype=np.float32)
    w_fsmn = np.asarray(w_fsmn, dtype=np.float32)

    assert x.shape == (B, T, F) and mask.shape == (B, 1, T)

    # mask must be a shared valid-prefix across the batch (as in batched ASR)
    m = mask.reshape(B, T)
    nv = int(round(float(m[0].sum())))
    expect = np.zeros(T, np.float32)
    expect[:nv] = 1.0
    if not np.all(m == expect[None, :]):
        raise NotImplementedError("kernel supports shared prefix masks only")
    nv = max(128, min(T, nv))

    use_bqkv = bool(np.any(b_qkv))
    use_bout = bool(np.any(b_out))
    nc = _get_nc(nv, use_bqkv, use_bout)

    nvt = _ceil_div(nv, 128)
    wdiag = _make_wdiag(w_fsmn)
    wfsmn_t = np.ascontiguousarray(
        w_fsmn.reshape(4, 128, KERNEL).transpose(1, 0, 2)
    )
    xT16 = [
        np.ascontiguousarray(
            x[c * NB:(c + 1) * NB].transpose(0, 2, 1).astype(np.float16)
        )
        for c in range(N_CORES)
    ]
    wqkv16 = np.ascontiguousarray(w_qkv.astype(np.float16))
    wout16 = np.ascontiguousarray(w_out.astype(np.float16))
    in_maps = []
    for c in range(N_CORES):
        im = {
            "xT": xT16[c],
            "wqkv": wqkv16,
            "wout": wout16,
            "wdiag": wdiag,
            "wfsmn": wfsmn_t,
        }
        if use_bqkv:
            im["bqkv"] = np.ascontiguousarray(b_qkv[None, :])
        if use_bout:
            im["bout"] = np.ascontiguousarray(b_out.reshape(4, 128).T)
        if nv - (nvt - 1) * 128 != 128:
            vcol = np.zeros((128, 1), np.float32)
            vcol[: nv - (nvt - 1) * 128] = 1.0
            im["vcol"] = vcol
        in_maps.append(im)

    global _last_in_maps
    _last_in_maps = in_maps
    res = run_bass_kernel_spmd(nc, in_maps, list(range(N_CORES)))
    out = np.empty((B, T, F), np.float32)
    for c in range(N_CORES):
        oT = res.results[c]["outT"]  # (NB, F, T)
        for i in range(NB):
            out[c * NB + i] = oT[i].T
    return out

